# revision 15
# baseline (speedup 1.0000x reference)
"""Trainium2 Bass kernel for a 6-layer post-LN transformer encoder.

Sharding: data-parallel over batch — 8 batch elements, one per NeuronCore.
No collectives. Each core runs the full encoder on its [512, 512] slice.

Layout strategy (per core):
  - residual stream x kept natural [tok, dm] in fp32 (4 tiles of [128, 512])
  - matmul inputs in bf16; fp32 accumulation in PSUM
  - x cast to bf16 and DMA-transposed to xT [dm, tok] where projections need it
  - attention computed in [k, q] orientation:
      energyT[k, q] = kT.T-slices @ qT   (2 heads packed in PE row groups, K=64)
      expT = exp(scale * energyT)        (ScalarE, no max subtraction; |e*scale|<4)
      denom[q] = ones.T @ expT           (M=1 matmuls, 4 heads packed in col groups)
      ctxT[d, q] = v-slices.T @ expT     (2 heads packed in PE col groups)
      normalize: reciprocal (VectorE) + partition_broadcast (GPSIMD) + fused mult
  - LayerNorm in natural layout: bn_stats/bn_aggr; rsqrt via exp(-0.5*ln(var+eps))
    (Ln and Exp share one ACT table set, avoiding table-switch stalls)
"""

import numpy as np
import ml_dtypes
from contextlib import ExitStack

import concourse.bass as bass
import concourse.tile as tile
from concourse import bacc, mybir
from concourse.bass_utils import run_bass_kernel_spmd

F32 = mybir.dt.float32
BF16 = mybir.dt.bfloat16
AF = mybir.ActivationFunctionType
ALU = mybir.AluOpType

D, NL, H, DFF, DIN = 512, 6, 8, 2048, 64
B, S = 8, 512
DH = D // H          # 64
P = 128
NT = S // P          # 4 token tiles
KD = D // P          # 4 model-dim tiles
KF = DFF // P        # 16 ff tiles
EPS = 1e-5
SCALE = float(1.0 / np.sqrt(D))
SQD = float(np.sqrt(D))


def _pe_table(seq_len, d_model):
    pos = np.arange(seq_len, dtype=np.float32)[:, None]
    div = np.exp(np.arange(0, d_model, 2, dtype=np.float32) * (-np.log(10000.0) / d_model))
    pe = np.zeros((seq_len, d_model), dtype=np.float32)
    pe[:, 0::2] = np.sin(pos * div)
    pe[:, 1::2] = np.cos(pos * div)
    return pe


def build(repeat=1, probe=None):
    """Builds the Bass program. probe: dump an intermediate and stop early."""
    nc = bacc.Bacc("TRN2", target_bir_lowering=False, debug=False, num_devices=8)

    # ---- DRAM tensors ----
    srcT = nc.dram_tensor("srcT", [DIN, S], BF16, kind="ExternalInput").ap()
    finw1 = nc.dram_tensor("finw1", [DIN, DFF], BF16, kind="ExternalInput").ap()
    finw2 = nc.dram_tensor("finw2", [DFF, D], BF16, kind="ExternalInput").ap()
    pe_fold = nc.dram_tensor("pe_fold", [S, D], F32, kind="ExternalInput").ap()
    wq = nc.dram_tensor("wq", [NL, D, D], BF16, kind="ExternalInput").ap()
    wk = nc.dram_tensor("wk", [NL, D, D], BF16, kind="ExternalInput").ap()
    wv = nc.dram_tensor("wv", [NL, D, D], BF16, kind="ExternalInput").ap()
    wo = nc.dram_tensor("wo", [NL, D, D], BF16, kind="ExternalInput").ap()
    ffw1 = nc.dram_tensor("ffw1", [NL, D, DFF], BF16, kind="ExternalInput").ap()
    ffw2 = nc.dram_tensor("ffw2", [NL, DFF, D], BF16, kind="ExternalInput").ap()
    out_dram = nc.dram_tensor("out", [S, D], F32, kind="ExternalOutput").ap()

    with tile.TileContext(nc) as tc, ExitStack() as ctx:
        # ---- pools ----
        wpool = ctx.enter_context(tc.tile_pool(name="w", bufs=1))
        apool = ctx.enter_context(tc.tile_pool(name="a", bufs=1))
        cpool = ctx.enter_context(tc.tile_pool(name="c", bufs=1))
        psum = ctx.enter_context(tc.tile_pool(name="ps", bufs=1, space="PSUM"))

        # constants (allocated once)
        eps_col = cpool.tile([P, 1], F32, tag="eps", bufs=1)
        nc.vector.memset(eps_col[:], EPS)

        def dump(tiles):
            # DMA tiles (cast to f32 if needed) into out_dram rows, then stop
            row = 0
            for ti, tl in enumerate(tiles):
                if len(tl.shape) > 2:
                    tl = tl.rearrange("p a b -> p (a b)")
                pr = tl.shape[0]
                fr = min(int(tl.shape[1]), D)
                if tl.dtype != F32:
                    sc = apool.tile([P, D], F32, tag="probef32", bufs=2, name=f"prb{ti}")
                    nc.vector.tensor_copy(sc[:pr, :fr], tl[:, :fr])
                    tl = sc
                nc.sync.dma_start(out_dram[row:row + pr, :fr], tl[:pr, :fr])
                row += pr

        def body():
            # ====================== input FFN ======================
            srcT_sb = apool.tile([DIN, S], BF16, tag="srcT", bufs=1)
            nc.sync.dma_start(srcT_sb[:], srcT)
            fw1_sb = apool.tile([DIN, DFF], BF16, tag="fw1", bufs=1)
            nc.sync.dma_start(fw1_sb[:], finw1)
            fw2_sb = wpool.tile([P, KF, D], BF16, tag="ffw2", bufs=2)
            nc.sync.dma_start(fw2_sb[:], finw2.rearrange("(kt p) n -> p kt n", p=P))
            pe_sb = [apool.tile([P, D], F32, tag="pe", bufs=NT, name=f"pe{t}") for t in range(NT)]
            for t in range(NT):
                nc.sync.dma_start(pe_sb[t][:], pe_fold[t * P:(t + 1) * P, :])

            h1T = []
            for m in range(KF):
                hp = psum.tile([P, S], F32, tag="acc", bufs=2)
                nc.tensor.matmul(hp[:], fw1_sb[:, m * P:(m + 1) * P], srcT_sb[:],
                                 start=True, stop=True)
                ht = apool.tile([P, S], BF16, tag="h1T", bufs=KF + 1)
                nc.scalar.activation(ht[:], hp[:], AF.Relu)
                h1T.append(ht)

            x = []
            for t in range(NT):
                xp = psum.tile([P, D], F32, tag="acc", bufs=2)
                for kt in range(KF):
                    nc.tensor.matmul(xp[:], h1T[kt][:, t * P:(t + 1) * P],
                                     fw2_sb[:, kt, :],
                                     start=(kt == 0), stop=(kt == KF - 1))
                xt = apool.tile([P, D], F32, tag="x", bufs=12)
                # x = psum * sqrt(D) + (pe + fin_b2*sqrt(D))
                nc.vector.scalar_tensor_tensor(xt[:], xp[:], SQD, pe_sb[t][:],
                                               op0=ALU.mult, op1=ALU.add)
                x.append(xt)

            if probe == "fin":
                return dump(x)
            # ====================== encoder layers ======================
            for i in range(NL):
                # ---- weight loads (early emission lets DMA run ahead) ----
                wq_sb = wpool.tile([P, KD, D], BF16, tag="wqkvo", bufs=6)
                nc.sync.dma_start(wq_sb[:], wq[i].rearrange("(kt p) n -> p kt n", p=P))
                wk_sb = wpool.tile([P, KD, D], BF16, tag="wqkvo", bufs=6)
                nc.sync.dma_start(wk_sb[:], wk[i].rearrange("(kt p) n -> p kt n", p=P))
                wv_sb = wpool.tile([P, KD, D], BF16, tag="wqkvo", bufs=6)
                nc.sync.dma_start(wv_sb[:], wv[i].rearrange("(kt p) n -> p kt n", p=P))
                # wo in [dh, head, n] layout so per-head K=64 o-proj matmuls
                # read lhsT and rhs from the same partition range (0-63)
                wo_sb = wpool.tile([DH, H, D], BF16, tag="wo64", bufs=1)
                nc.sync.dma_start(wo_sb[:], wo[i].rearrange("(h dh) n -> dh h n", dh=DH))
                f1_sb = [wpool.tile([P, DFF], BF16, tag="ffw1", bufs=5, name=f"f1_{i}_{kt}")
                         for kt in range(KD)]
                for kt in range(KD):
                    nc.sync.dma_start(f1_sb[kt][:], ffw1[i][kt * P:(kt + 1) * P, :])
                f2_sb = wpool.tile([P, KF, D], BF16, tag="ffw2", bufs=2)
                nc.sync.dma_start(f2_sb[:], ffw2[i].rearrange("(kt p) n -> p kt n", p=P))

                # ---- cast x to bf16 and transpose ----
                xT = [apool.tile([P, S], BF16, tag="xT", bufs=2 * KD, name=f"xT{i}_{j}") for j in range(KD)]
                for t in range(NT):
                    xbf = apool.tile([P, D], BF16, tag="xbf", bufs=NT + 1)
                    nc.vector.tensor_copy(xbf[:], x[t][:])
                    for j in range(KD):
                        nc.sync.dma_start_transpose(
                            xT[j][:, t * P:(t + 1) * P], xbf[:, j * P:(j + 1) * P])

                if probe == "xT" and i == 0:
                    return dump(xT)
                # ---- q/k/v projections ----
                qT, kT = [], []
                for w_sb, dst in ((wq_sb, qT), (wk_sb, kT)):
                    for m in range(KD):
                        pp = psum.tile([P, S], F32, tag="acc", bufs=2)
                        for kt in range(KD):
                            nc.tensor.matmul(pp[:], w_sb[:, kt, m * P:(m + 1) * P],
                                             xT[kt][:],
                                             start=(kt == 0), stop=(kt == KD - 1))
                        qt = apool.tile([P, S], BF16,
                                        tag="qT" if dst is qT else "kTt", bufs=KD + 1)
                        nc.vector.tensor_copy(qt[:], pp[:])
                        dst.append(qt)
                # v stored ones-augmented: [128 tok, head, 66] with column 64
                # set to 1.0 so lhsT = v_sb[:, h, 0:65] makes the ctxT matmul
                # also produce the softmax denominator in output row 64.
                v = []
                for t in range(NT):
                    pp = psum.tile([P, D], F32, tag="acc", bufs=2)
                    for kt in range(KD):
                        nc.tensor.matmul(pp[:], xT[kt][:, t * P:(t + 1) * P],
                                         wv_sb[:, kt, :],
                                         start=(kt == 0), stop=(kt == KD - 1))
                    vt = apool.tile([P, H, DH + 2], BF16, tag="v", bufs=NT + 1)
                    nc.vector.memset(vt[:, :, DH:DH + 2], 1.0)
                    nc.vector.tensor_copy(vt[:, :, 0:DH],
                                          pp.rearrange("p (h d) -> p h d", d=DH))
                    v.append(vt)

                if probe == "qT" and i == 0:
                    return dump(qT)
                if probe == "kT" and i == 0:
                    return dump(kT)
                if probe == "v" and i == 0:
                    return dump(v)
                # ---- attention ----
                ctxT = []  # 8 tiles [64, S] bf16, one per head
                for j in range(KD):  # head pair (2j, 2j+1)
                    expT = {}  # (hh, kc) -> bf16 [128, S] tile
                    for kc in range(NT):
                        for hh in range(2):
                            ep = psum.tile([P, S], F32, tag="e", bufs=6)
                            nc.tensor.matmul(
                                ep[:],
                                kT[j][hh * DH:(hh + 1) * DH, kc * P:(kc + 1) * P],
                                qT[j][hh * DH:(hh + 1) * DH, :],
                                start=True, stop=True)
                            ex = apool.tile([P, S], BF16, tag="expT", bufs=8)
                            nc.scalar.activation(ex[:], ep[:], AF.Exp, scale=SCALE)
                            expT[(hh, kc)] = ex
                    if probe == "expT0" and i == 0 and j == 0:
                        return dump([expT[(0, kc)] for kc in range(NT)])
                    for hh in range(2):
                        h = 2 * j + hh
                        # ctxT rows 0-63 plus denominator in row 64, one group
                        cp = psum.tile([DH + 1, S], F32, tag="acc", bufs=2,
                                       name=f"cp{i}_{h}")
                        for kc in range(NT):
                            nc.tensor.matmul(cp[:], v[kc][:, h, 0:DH + 1],
                                             expT[(hh, kc)][:],
                                             start=(kc == 0), stop=(kc == NT - 1))
                        rcp = apool.tile([DH + 1, S], F32, tag="rcp", bufs=2,
                                         name=f"rcp{i}_{h}")
                        nc.vector.reciprocal(rcp[DH:DH + 1, :], cp[DH:DH + 1, :])
                        # hop the row to partition 0: HW partition_broadcast
                        # reads the source tensor's partition 0 regardless of
                        # the AP's partition offset (sim honors the offset)
                        rcp0 = apool.tile([1, S], F32, tag="rcp0", bufs=2,
                                          name=f"rcp0_{i}_{h}")
                        nc.sync.dma_start(rcp0[0:1, :], rcp[DH:DH + 1, :])
                        rbc = apool.tile([DH, S], F32, tag="rbc", bufs=2,
                                         name=f"rbc{i}_{h}")
                        nc.gpsimd.partition_broadcast(rbc[:], rcp0[0:1, :],
                                                      channels=DH)
                        ct = apool.tile([DH, S], BF16, tag="ctxT", bufs=H + 1,
                                        name=f"ct{i}_{h}")
                        nc.vector.tensor_tensor(ct[:], cp[0:DH, :], rbc[:], ALU.mult)
                        ctxT.append(ct)
                        if probe is not None and probe.startswith("cpraw") and i == 0 and h == int(probe[5:]):
                            sc_ = apool.tile([P, S], F32, tag="probef32", bufs=2, name="cpc")
                            nc.vector.tensor_copy(sc_[:DH + 1, :], cp[:])
                            return dump([sc_])
                        if probe is not None and probe.startswith("rbcraw") and i == 0 and h == int(probe[6:]):
                            return dump([rbc])

                if probe == "ctx" and i == 0:
                    return dump(ctxT)
                # ---- output projection (per-head K=64) + residual ----
                xr = []
                for t in range(NT):
                    op = psum.tile([P, D], F32, tag="acc", bufs=2)
                    for h in range(H):
                        nc.tensor.matmul(op[:], ctxT[h][:, t * P:(t + 1) * P],
                                         wo_sb[:, h, :],
                                         start=(h == 0), stop=(h == H - 1))
                    xt = apool.tile([P, D], F32, tag="x", bufs=12)
                    nc.vector.tensor_tensor(xt[:], op[:], x[t][:], ALU.add)
                    xr.append(xt)

                if probe == "oproj" and i == 0:
                    return dump(xr)
                # ---- LN1 ----
                x = _layernorm(nc, apool, xr)

                if probe == "ln1" and i == 0:
                    return dump(x)
                # ---- FFN ----
                xT2 = [apool.tile([P, S], BF16, tag="xT", bufs=2 * KD, name=f"xU{i}_{j}") for j in range(KD)]
                for t in range(NT):
                    xbf = apool.tile([P, D], BF16, tag="xbf", bufs=NT + 1)
                    nc.vector.tensor_copy(xbf[:], x[t][:])
                    for j in range(KD):
                        nc.sync.dma_start_transpose(
                            xT2[j][:, t * P:(t + 1) * P], xbf[:, j * P:(j + 1) * P])
                h1 = []
                for m in range(KF):
                    hp = psum.tile([P, S], F32, tag="acc", bufs=2)
                    for kt in range(KD):
                        nc.tensor.matmul(hp[:], f1_sb[kt][:, m * P:(m + 1) * P],
                                         xT2[kt][:],
                                         start=(kt == 0), stop=(kt == KD - 1))
                    ht = apool.tile([P, S], BF16, tag="h1T", bufs=KF + 1)
                    nc.scalar.activation(ht[:], hp[:], AF.Relu)
                    h1.append(ht)
                xr2 = []
                for t in range(NT):
                    fp = psum.tile([P, D], F32, tag="acc", bufs=2)
                    for kt in range(KF):
                        nc.tensor.matmul(fp[:], h1[kt][:, t * P:(t + 1) * P],
                                         f2_sb[:, kt, :],
                                         start=(kt == 0), stop=(kt == KF - 1))
                    xt = apool.tile([P, D], F32, tag="x", bufs=12)
                    nc.vector.tensor_tensor(xt[:], fp[:], x[t][:], ALU.add)
                    xr2.append(xt)

                if probe == "ffn" and i == 0:
                    return dump(xr2)
                # ---- LN2 ----
                x = _layernorm(nc, apool, xr2)

            # ---- store output ----
            for t in range(NT):
                nc.sync.dma_start(out_dram[t * P:(t + 1) * P, :], x[t][:])

        def _layernorm(nc, apool, xin):  # noqa: uses eps_col from enclosing scope
            mv = apool.tile([P, 2 * NT], F32, tag="mv", bufs=2)
            for t in range(NT):
                st6 = apool.tile([P, 6], F32, tag="st6", bufs=NT + 1)
                nc.vector.bn_stats(st6[:], xin[t][:])
                nc.vector.bn_aggr(mv[:, 2 * t:2 * t + 2], st6[:])
            lnv = apool.tile([P, NT], F32, tag="lnv", bufs=2)
            # ln(var + eps); var at odd columns of mv
            nc.scalar.activation(lnv[:], mv[:, 1:2 * NT:2], AF.Ln, bias=eps_col[:])
            rs = apool.tile([P, NT], F32, tag="rs", bufs=2)
            nc.scalar.activation(rs[:], lnv[:], AF.Exp, scale=-0.5)
            xout = []
            for t in range(NT):
                xt = apool.tile([P, D], F32, tag="x", bufs=12)
                nc.vector.tensor_scalar(xt[:], xin[t][:],
                                        mv[:, 2 * t:2 * t + 1], rs[:, t:t + 1],
                                        op0=ALU.subtract, op1=ALU.mult)
                xout.append(xt)
            return xout

        if repeat == 1:
            body()
        else:
            with tc.For_i(0, repeat, 1):
                body()

    nc.finalize()
    return nc


_CACHE = {}


def _get_nc(repeat=1, probe=None):
    key = (repeat, probe)
    if key not in _CACHE:
        _CACHE[key] = build(repeat, probe)
    return _CACHE[key]


def prepare_in_maps(inputs):
    """Host-side prep: dtype casts, transposes, PE-table fold. Returns per-core in_maps."""
    bf = ml_dtypes.bfloat16
    g = {k: np.asarray(v) for k, v in inputs.items()}

    # This kernel build skips bias/LN-affine ops that are identity for the
    # reference initialization; verify that assumption on the actual inputs.
    for name in ("fin_b1", "bq", "bk", "bv", "bo", "ffb1", "ffb2", "n1_b", "n2_b"):
        if np.any(g[name]):
            raise NotImplementedError(f"nonzero {name} not supported by this build")
    for name in ("n1_s", "n2_s"):
        if not np.all(g[name] == 1.0):
            raise NotImplementedError(f"non-unit {name} not supported by this build")

    pe_fold = (_pe_table(S, D) + np.asarray(g["fin_b2"], np.float32) * SQD).astype(np.float32)
    shared = {
        "finw1": g["fin_w1"].astype(bf),
        "finw2": g["fin_w2"].astype(bf),
        "pe_fold": pe_fold,
        "wq": g["wq"].astype(bf), "wk": g["wk"].astype(bf),
        "wv": g["wv"].astype(bf), "wo": g["wo"].astype(bf),
        "ffw1": g["ffw1"].astype(bf), "ffw2": g["ffw2"].astype(bf),
    }
    src = np.asarray(g["source"], np.float32)  # [B, S, DIN]
    in_maps = []
    for c in range(B):
        m = dict(shared)
        m["srcT"] = np.ascontiguousarray(src[c].T).astype(bf)
        in_maps.append(m)
    return in_maps


def kernel(**inputs):
    nc = _get_nc(repeat=1)
    in_maps = prepare_in_maps(inputs)
    res = run_bass_kernel_spmd(nc, in_maps, core_ids=list(range(8)))
    return np.stack([res.results[c]["out"] for c in range(B)], axis=0)
